# revision 13
# baseline (speedup 1.0000x reference)
"""Two-layer GCN (PyG GCNConv semantics) on 8 Trainium2 NeuronCores.

Math: out = Ahat @ relu(Ahat @ (X@W1) + b1) @ W2 + b2, with
Ahat = D^-1/2 (A + I) D^-1/2.  Edge norm dinv[src]*dinv[dst] is folded
into per-edge one-hot aggregation matrices (segmented matmul on the PE,
edges sorted by dst, 128-edge tiles).  Self-loops are NOT materialized
as edges: their diagonal contribution dinv[d]^2 * row[d] is added per
128-dst block with one dense matmul against host-prescaled tensors.

Layer 1 defers W1: aggregate raw X rows (gathered by edge src via gpsimd
dma_gather), then per 128-dst block project the aggregate through W1
(+b1, relu) AND W2, storing t2 = relu(.)@W2 (64 wide, bf16).

Both layers split edges by src PARITY, giving one shared tile layout:
the int16 gather index is src>>1 for both (fits 25000 < 32768).
Layer 1 gathers 256B X rows through an even/odd strided view
(elem_step=256 elems = 2 rows); layer 2 gathers 256B PAIR rows of the
64-wide t2 table (the parity picks the rhs column half).

Sharding: destination nodes split across 8 cores (6250 each); one
AllGather shares the 64-wide layer-2 source table.
"""

import sys

import numpy as np

try:
    import concourse.bass as bass  # noqa: F401
except ImportError:
    sys.path.insert(0, "/opt/trn_rl_repo")

from contextlib import ExitStack

import ml_dtypes

import concourse.bass as bass
import concourse.tile as tile
from concourse import bacc, mybir
from concourse.bass_utils import run_bass_kernel_spmd

BF16 = ml_dtypes.bfloat16

# debug ablation: 0 = no dma_gather + no collective, 1 = gather + no collective,
# 2 = full kernel
ABLATE = 2

N = 50000
E = 800000
FIN = 128
HID = 128
FOUT = 64
NCORES = 8
NSH = N // NCORES  # 6250 destination nodes per core
BLK = 128  # dst block (psum window)
NBLK = (NSH + BLK - 1) // BLK  # 49
NSHP = NBLK * BLK  # 6272, padded
SBW = 4  # dst blocks per superblock
NSB = (NBLK + SBW - 1) // SBW  # 13
GCH = 8  # gather chunk: 8 tiles = 1024 idx (SWDGE ring limit, ucode-fixed)
NQ = 4  # swdge queues, round-robined across gather calls
# collective chunks (row ranges of t2loc), aligned to superblock boundaries
CCHUNKS = [(0, 2048), (2048, 3584), (3584, 5120), (5120, NSH)]
# t2full/xrow row permutation: (chunk, core, row) order makes each chunk's
# AllGather output a contiguous slice.  newpos preserves row parity.
_CHB = np.cumsum([0] + [NCORES * (r1 - r0) for r0, r1 in CCHUNKS])


def _newpos(n):
    c, local = n // NSH, n % NSH
    pos = np.zeros_like(n)
    for q, (r0, r1) in enumerate(CCHUNKS):
        m = (local >= r0) & (local < r1)
        pos[m] = _CHB[q] + c[m] * (r1 - r0) + (local[m] - r0)
    return pos


def _layout(tiles):
    """Static program layout from per-(block, parity) tile counts.

    Data order: (sb, parity, block); gather segments keyed (sb, parity).
    Returns (TT, tile_base[NBLK][2], seg: (sb, par) -> (tile0, ntiles)).
    """
    tile_base = np.zeros((NBLK, 2), dtype=np.int64)
    seg = {}
    pos = 0
    for sb in range(NSB):
        blocks = range(sb * SBW, min((sb + 1) * SBW, NBLK))
        for h in (0, 1):
            seg_start = pos
            for b in blocks:
                tile_base[b][h] = pos
                pos += int(tiles[b][h])
            seg[(sb, h)] = (seg_start, pos - seg_start)
    return int(pos), tile_base, seg


def _prep(edge_index):
    """Shared (both layers) tile layout + per-core meta/idx.

    Returns (tiles, TT, tile_base, seg, meta, idx, dinv).
    """
    s_all = np.asarray(edge_index[0], dtype=np.int64)
    d_all = np.asarray(edge_index[1], dtype=np.int64)
    deg = (np.bincount(d_all, minlength=N) + 1).astype(np.float64)
    dinv = (1.0 / np.sqrt(deg)).astype(np.float32)

    core = d_all // NSH
    local = d_all % NSH
    block = local // BLK
    sbk = block // SBW
    par = (s_all & 1).astype(np.int64)

    cidx = (core * NBLK + block) * 2 + par
    cnt = np.bincount(cidx, minlength=NCORES * NBLK * 2).reshape(NCORES, NBLK, 2)
    tiles = ((cnt + BLK - 1) // BLK).max(axis=0)  # [NBLK, 2] max over cores

    TT, tile_base, seg = _layout(tiles)
    S = TT * BLK

    order = np.lexsort((local, block, par, sbk, core))
    s_s = s_all[order]
    d_s = d_all[order]
    core_s = core[order]
    block_s = block[order]
    h_s = par[order]

    gid = (core_s * NBLK + block_s) * 2 + h_s
    change = np.r_[True, gid[1:] != gid[:-1]]
    gstart = np.maximum.accumulate(np.where(change, np.arange(len(gid)), 0))
    rank = np.arange(len(gid)) - gstart
    slot = tile_base[block_s, h_s] * BLK + rank  # per-core slot in [0, S)

    src_loc = (_newpos(s_s & ~1) >> 1).astype(np.int16)  # permuted pair idx
    dst_loc = (d_s % NSH - block_s * BLK).astype(np.float32)  # 0..127
    norm = dinv[d_s] * dinv[s_s]

    seg_slot0_tbl = np.zeros(TT, dtype=np.int64)  # per tile: slot0 of its seg
    for _key, (t0, nt) in seg.items():
        seg_slot0_tbl[t0 : t0 + nt] = t0 * BLK

    meta_np = np.zeros((NCORES, 128, TT, 2), dtype=np.float32)
    idx_np = np.zeros((NCORES, 128, S // 16), dtype=np.int16)
    for c in range(NCORES):
        m = core_s == c
        sl = slot[m]
        tt = sl // BLK
        pp = sl % BLK
        meta_np[c, pp, tt, 0] = dst_loc[m]
        meta_np[c, pp, tt, 1] = norm[m]
        seg0 = seg_slot0_tbl[tt]
        j = sl - seg0
        col = seg0 // 16 + j // 16
        row = j % 16
        v = src_loc[m]
        for g in range(8):  # replicate across the 8 gpsimd 16-partition groups
            idx_np[c, row + 16 * g, col] = v
    allpos = _newpos(np.arange(N, dtype=np.int64))
    order_nodes = np.empty(N, dtype=np.int64)
    order_nodes[allpos] = np.arange(N, dtype=np.int64)
    return tiles, TT, tile_base, seg, meta_np, idx_np, dinv, order_nodes


def _build(prep):
    tiles, TT, tile_base, seg, _meta, _idx, _dinv, _ord = prep
    S = TT * BLK
    f32 = mybir.dt.float32
    bf16 = mybir.dt.bfloat16
    i16 = mybir.dt.int16
    AF = mybir.ActivationFunctionType
    OP = mybir.AluOpType

    nc = bacc.Bacc(
        "TRN2",
        target_bir_lowering=False,
        debug=False,
        num_devices=NCORES,
        num_swdge_queues=NQ,
    )
    xrow = nc.dram_tensor("xrow", [N, FIN], bf16, kind="ExternalInput")
    w1 = nc.dram_tensor("w1", [128, HID], bf16, kind="ExternalInput")
    w2 = nc.dram_tensor("w2", [128, FOUT], bf16, kind="ExternalInput")
    b1c = nc.dram_tensor("b1c", [128, 1], f32, kind="ExternalInput")
    b2r = nc.dram_tensor("b2r", [128, FOUT], f32, kind="ExternalInput")
    iot = nc.dram_tensor("iot", [128, BLK], bf16, kind="ExternalInput")
    meta = nc.dram_tensor("meta", [128, TT, 2], f32, kind="ExternalInput")
    idxt = nc.dram_tensor("idx", [128, S // 16], i16, kind="ExternalInput")
    xself = nc.dram_tensor("xself", [128, NSHP], bf16, kind="ExternalInput")
    ident = nc.dram_tensor("ident", [128, BLK], bf16, kind="ExternalInput")
    diag2 = nc.dram_tensor("diag2", [128, NBLK, BLK], bf16, kind="ExternalInput")
    outp = nc.dram_tensor("out", [NSH, FOUT], f32, kind="ExternalOutput")

    with tile.TileContext(nc) as tc, ExitStack() as ctx:
        const = ctx.enter_context(tc.tile_pool(name="const", bufs=1))
        dram = ctx.enter_context(tc.tile_pool(name="dram", bufs=1, space="DRAM"))
        gp1 = ctx.enter_context(tc.tile_pool(name="g1", bufs=2))
        gp2 = ctx.enter_context(tc.tile_pool(name="g2", bufs=2))
        tpp = ctx.enter_context(tc.tile_pool(name="tp", bufs=40))
        evp = ctx.enter_context(tc.tile_pool(name="ev", bufs=4))
        psa = ctx.enter_context(tc.tile_pool(name="psa", bufs=3, space="PSUM"))
        psb = ctx.enter_context(tc.tile_pool(name="psb", bufs=1, space="PSUM"))
        psc = ctx.enter_context(tc.tile_pool(name="psc", bufs=1, space="PSUM"))
        psd = ctx.enter_context(tc.tile_pool(name="psd", bufs=3, space="PSUM"))

        def cload(ap, shape, dtype, tag):
            t = const.tile(shape, dtype, tag=tag)
            nc.sync.dma_start(t[:], ap)
            return t

        w1_sb = cload(w1[:, :], [128, HID], bf16, "w1")
        w2_sb = cload(w2[:, :], [128, FOUT], bf16, "w2")
        b1_sb = cload(b1c[:, :], [128, 1], f32, "b1")
        b2_sb = cload(b2r[:, :], [128, FOUT], f32, "b2")
        iota_sb = cload(iot[:, :], [128, BLK], bf16, "iota")
        meta_sb = cload(meta[:, :, :], [128, TT, 2], f32, "meta")
        idx_sb = cload(idxt[:, :], [128, S // 16], i16, "idx")
        xself_sb = cload(xself[:, :], [128, NSHP], bf16, "xself")
        ident_sb = cload(ident[:, :], [128, BLK], bf16, "ident")
        diag_sb = cload(diag2[:, :, :], [128, NBLK, BLK], bf16, "diag2")

        t2loc = dram.tile([NSHP, FOUT], bf16, tag="t2loc")
        t2full = dram.tile([N, FOUT], bf16, tag="t2full")
        # pair view for the layer-2 gather: row p = nodes (2p, 2p+1), 256B
        t2pair = t2full[:, :].rearrange("(p two) f -> p (two f)", two=2)
        # even/odd views of X for the layer-1 gather (elem_step = 2 rows)
        xpar = xrow[:, :].rearrange("(p two) f -> p two f", two=2)

        # zero-fill the padded t2loc tail (layer-2 diag matmul reads it)
        zt = evp.tile([128, FOUT], bf16, tag="zt")
        nc.vector.memset(zt[:], 0)
        nc.sync.dma_start(t2loc[NSH:NSHP, :], zt[0 : NSHP - NSH, :])

        qctr = [0]

        def gather(g, view, t0, ntl, elem, elem_step=None):
            for q0 in range(0, ntl, GCH):
                qn = min(GCH, ntl - q0)
                c0 = (t0 + q0) * 8  # idx columns (tile*128/16)
                nc.gpsimd.dma_gather(
                    out_ap=g[:, q0 : q0 + qn, :],
                    in_ap=view,
                    idxs_ap=idx_sb[:, c0 : c0 + qn * 8],
                    num_idxs=qn * 128,
                    num_idxs_reg=qn * 128,
                    elem_size=elem,
                    elem_step=elem_step,
                    queue_num=qctr[0] % NQ,
                )
                qctr[0] += 1

        def onehot(t):
            tp = tpp.tile([128, BLK], bf16, tag="tp")
            nc.vector.tensor_scalar(
                out=tp[:],
                in0=iota_sb[:],
                scalar1=meta_sb[:, t, 0:1],
                scalar2=meta_sb[:, t, 1:2],
                op0=OP.is_equal,
                op1=OP.mult,
            )
            return tp

        # ---- Layer 1: aggregate X, project W1 (+b1, relu) and W2 per block
        for sb in range(NSB):
            blocks = list(range(sb * SBW, min((sb + 1) * SBW, NBLK)))
            gt = {}
            for h in (0, 1):
                t0, ntl = seg[(sb, h)]
                if ntl == 0:
                    continue
                g = gp1.tile([128, ntl, FIN], bf16, tag=f"g1{h}")
                if ABLATE >= 1:
                    gather(g, xpar[:, h, :], t0, ntl, FIN, elem_step=2 * FIN)
                else:
                    nc.vector.memset(g[:], 0)
                gt[h] = g
            for b in blocks:
                r0 = b * BLK
                r1 = min(NSH, r0 + BLK)
                ps2 = psa.tile([128, BLK], f32, tag="psa")  # [xfeat, dst]
                # self-loop diagonal: identity lhsT copies the prescaled block
                nc.tensor.matmul(
                    ps2[:],
                    lhsT=ident_sb[:],
                    rhs=xself_sb[:, r0 : r0 + BLK],
                    start=True,
                    stop=False,
                )
                nmm = sum(int(tiles[b][h]) for h in (0, 1))
                done = 0
                for h in (0, 1):
                    seg_t0 = seg[(sb, h)][0]
                    for k in range(int(tiles[b][h])):
                        t = int(tile_base[b][h]) + k
                        gofs = t - seg_t0
                        tp = onehot(t)
                        done += 1
                        nc.tensor.matmul(
                            ps2[:],
                            lhsT=gt[h][:, gofs, :],
                            rhs=tp[:],
                            start=False,
                            stop=done == nmm,
                        )
                a2 = evp.tile([128, BLK], bf16, tag="a2")
                nc.scalar.activation(a2[:], ps2[:], AF.Copy)
                poT = psb.tile([128, BLK], f32, tag="psb")  # [hid, dst]
                nc.tensor.matmul(
                    poT[:], lhsT=w1_sb[:], rhs=a2[:], start=True, stop=True
                )
                hbT = evp.tile([128, BLK], bf16, tag="hbT")
                nc.scalar.activation(hbT[:], poT[:], AF.Relu, bias=b1_sb[:, 0:1])
                t2w = psc.tile([128, FOUT], f32, tag="psc")  # [dst, fout]
                nc.tensor.matmul(
                    t2w[:], lhsT=hbT[:], rhs=w2_sb[:], start=True, stop=True
                )
                t2e = evp.tile([128, FOUT], bf16, tag="t2e")
                nc.scalar.activation(t2e[:], t2w[:], AF.Copy)
                nc.sync.dma_start(t2loc[r0:r1, :], t2e[0 : r1 - r0, :])

        # ---- share the 64-wide t2 table (chunked to overlap layer 1);
        # t2full rows are in (chunk, core, row) order so each chunk's
        # output is one contiguous slice.
        for q, (r0, r1) in enumerate(CCHUNKS):
            g0 = int(_CHB[q])
            g1 = int(_CHB[q + 1])
            if ABLATE >= 2:
                nc.gpsimd.collective_compute(
                    "AllGather",
                    mybir.AluOpType.bypass,
                    replica_groups=[list(range(NCORES))],
                    ins=[t2loc[r0:r1, :].opt()],
                    outs=[t2full[g0:g1, :].opt()],
                )
            else:
                nc.sync.dma_start(t2full[g0 : g0 + (r1 - r0), :], t2loc[r0:r1, :])

        # ---- Layer 2: aggregate t2 pair rows (self diag via dense block), +b2
        for sb in range(NSB):
            blocks = list(range(sb * SBW, min((sb + 1) * SBW, NBLK)))
            gt = {}
            for h in (0, 1):
                t0, ntl = seg[(sb, h)]
                if ntl == 0:
                    continue
                g = gp2.tile([128, ntl, 2 * FOUT], bf16, tag=f"g2{h}")
                if ABLATE >= 1:
                    gather(g, t2pair, t0, ntl, 2 * FOUT)
                else:
                    nc.vector.memset(g[:], 0)
                gt[h] = g
            for b in blocks:
                r0 = b * BLK
                r1 = min(NSH, r0 + BLK)
                t2s = evp.tile([128, FOUT], bf16, tag="t2s")
                nc.sync.dma_start(t2s[:], t2loc[r0 : r0 + BLK, :])
                ps = psd.tile([128, FOUT], f32, tag="psd")  # [dst, fout]
                nc.tensor.matmul(
                    ps[:], lhsT=diag_sb[:, b, :], rhs=t2s[:], start=True, stop=False
                )
                nmm = sum(int(tiles[b][h]) for h in (0, 1))
                done = 0
                for h in (0, 1):
                    for k in range(int(tiles[b][h])):
                        t = int(tile_base[b][h]) + k
                        gofs = t - seg[(sb, h)][0]
                        tp = onehot(t)
                        done += 1
                        nc.tensor.matmul(
                            ps[:],
                            lhsT=tp[:],
                            rhs=gt[h][:, gofs, h * FOUT : (h + 1) * FOUT],
                            start=False,
                            stop=done == nmm,
                        )
                oo = evp.tile([128, FOUT], f32, tag="oo")
                nc.vector.tensor_add(oo[:], ps[:], b2_sb[:])
                nc.sync.dma_start(outp[r0:r1, :], oo[0 : r1 - r0, :])

    nc.finalize()
    return nc


def _in_maps(x, W1, b1, W2, b2, prep):
    tiles, TT, tile_base, seg, meta_np, idx_np, dinv, order_nodes = prep
    xf = np.asarray(x, np.float32)
    xr = np.ascontiguousarray(xf[order_nodes].astype(BF16))
    w1b = np.asarray(W1, np.float32).astype(BF16)
    w2b = np.asarray(W2, np.float32).astype(BF16)
    b1col = np.asarray(b1, np.float32).reshape(128, 1).copy()
    b2rep = np.broadcast_to(np.asarray(b2, np.float32), (128, FOUT)).copy()
    iota = np.broadcast_to(np.arange(BLK, dtype=np.float32), (128, BLK)).astype(BF16)
    shared = {
        "xrow": xr,
        "w1": w1b,
        "w2": w2b,
        "b1c": b1col,
        "b2r": b2rep,
        "iot": np.ascontiguousarray(iota),
        "ident": np.ascontiguousarray(np.eye(BLK, dtype=np.float32).astype(BF16)),
    }
    d2 = (dinv.astype(np.float64) ** 2).astype(np.float32)
    maps = []
    for c in range(NCORES):
        lo, hi = c * NSH, (c + 1) * NSH
        xs = np.zeros((128, NSHP), dtype=BF16)
        xs[:, :NSH] = (xf[lo:hi] * d2[lo:hi, None]).T.astype(BF16)
        dvals = np.zeros(NSHP, np.float32)
        dvals[:NSH] = d2[lo:hi]
        dg = np.zeros((128, NBLK, BLK), dtype=np.float32)
        for b in range(NBLK):
            dg[np.arange(BLK), b, np.arange(BLK)] = dvals[b * BLK : (b + 1) * BLK]
        maps.append(
            dict(
                shared,
                meta=np.ascontiguousarray(meta_np[c]),
                idx=np.ascontiguousarray(idx_np[c]),
                xself=xs,
                diag2=np.ascontiguousarray(dg.astype(BF16)),
            )
        )
    return maps


def kernel(x, edge_index, W1, b1, W2, b2):
    prep = _prep(edge_index)
    nc = _build(prep)
    in_maps = _in_maps(x, W1, b1, W2, b2, prep)
    res = run_bass_kernel_spmd(nc, in_maps, core_ids=list(range(NCORES)), trace=False)
    out = np.concatenate(
        [res.results[c]["out"].astype(np.float32) for c in range(NCORES)], axis=0
    )
    return out


# revision 14
# speedup vs baseline: 1.0870x; 1.0870x over previous
"""Two-layer GCN (PyG GCNConv semantics) on 8 Trainium2 NeuronCores.

Math: out = Ahat @ relu(Ahat @ (X@W1) + b1) @ W2 + b2, with
Ahat = D^-1/2 (A + I) D^-1/2.  Edge norm dinv[src]*dinv[dst] is folded
into per-edge one-hot aggregation matrices (segmented matmul on the PE,
edges sorted by dst, 128-edge tiles).  Self-loops are NOT materialized
as edges: their diagonal contribution dinv[d]^2 * row[d] is added per
128-dst block with one dense matmul against host-prescaled tensors.

Layer 1 defers W1: aggregate raw X rows (gathered by edge src via gpsimd
dma_gather), then per 128-dst block project the aggregate through W1
(+b1, relu) AND W2, storing t2 = relu(.)@W2 (64 wide, bf16).

Both layers split edges by src PARITY, giving one shared tile layout:
the int16 gather index is src>>1 for both (fits 25000 < 32768).
Layer 1 gathers 256B X rows through an even/odd strided view
(elem_step=256 elems = 2 rows); layer 2 gathers 256B PAIR rows of the
64-wide t2 table (the parity picks the rhs column half).

Sharding: destination nodes split across 8 cores (6250 each); one
AllGather shares the 64-wide layer-2 source table.
"""

import sys

import numpy as np

try:
    import concourse.bass as bass  # noqa: F401
except ImportError:
    sys.path.insert(0, "/opt/trn_rl_repo")

from contextlib import ExitStack

import ml_dtypes

import concourse.bass as bass
import concourse.tile as tile
from concourse import bacc, mybir
from concourse.bass_utils import run_bass_kernel_spmd

BF16 = ml_dtypes.bfloat16

# debug ablation: 0 = no dma_gather + no collective, 1 = gather + no collective,
# 2 = full kernel
ABLATE = 2

N = 50000
E = 800000
FIN = 128
HID = 128
FOUT = 64
NCORES = 8
NSH = N // NCORES  # 6250 destination nodes per core
BLK = 128  # dst block (psum window)
NBLK = (NSH + BLK - 1) // BLK  # 49
NSHP = NBLK * BLK  # 6272, padded
SBW = 4  # dst blocks per superblock
NSB = (NBLK + SBW - 1) // SBW  # 13
GCH = 8  # gather chunk: 8 tiles = 1024 idx (SWDGE ring limit, ucode-fixed)
NQ = 4  # swdge queues, round-robined across gather calls
# collective chunks (row ranges of t2loc), aligned to superblock boundaries
CCHUNKS = [(0, 2560), (2560, 4608), (4608, 5632), (5632, NSH)]
# t2full/xrow row permutation: (chunk, core, row) order makes each chunk's
# AllGather output a contiguous slice.  newpos preserves row parity.
_CHB = np.cumsum([0] + [NCORES * (r1 - r0) for r0, r1 in CCHUNKS])


def _newpos(n):
    c, local = n // NSH, n % NSH
    pos = np.zeros_like(n)
    for q, (r0, r1) in enumerate(CCHUNKS):
        m = (local >= r0) & (local < r1)
        pos[m] = _CHB[q] + c[m] * (r1 - r0) + (local[m] - r0)
    return pos


def _layout(tiles):
    """Static program layout from per-(block, parity) tile counts.

    Data order: (sb, parity, block); gather segments keyed (sb, parity).
    Returns (TT, tile_base[NBLK][2], seg: (sb, par) -> (tile0, ntiles)).
    """
    tile_base = np.zeros((NBLK, 2), dtype=np.int64)
    seg = {}
    pos = 0
    for sb in range(NSB):
        blocks = range(sb * SBW, min((sb + 1) * SBW, NBLK))
        for h in (0, 1):
            seg_start = pos
            for b in blocks:
                tile_base[b][h] = pos
                pos += int(tiles[b][h])
            seg[(sb, h)] = (seg_start, pos - seg_start)
    return int(pos), tile_base, seg


def _prep(edge_index):
    """Shared (both layers) tile layout + per-core meta/idx.

    Returns (tiles, TT, tile_base, seg, meta, idx, dinv).
    """
    s_all = np.asarray(edge_index[0], dtype=np.int64)
    d_all = np.asarray(edge_index[1], dtype=np.int64)
    deg = (np.bincount(d_all, minlength=N) + 1).astype(np.float64)
    dinv = (1.0 / np.sqrt(deg)).astype(np.float32)

    core = d_all // NSH
    local = d_all % NSH
    block = local // BLK
    sbk = block // SBW
    par = (s_all & 1).astype(np.int64)

    cidx = (core * NBLK + block) * 2 + par
    cnt = np.bincount(cidx, minlength=NCORES * NBLK * 2).reshape(NCORES, NBLK, 2)
    tiles = ((cnt + BLK - 1) // BLK).max(axis=0)  # [NBLK, 2] max over cores

    TT, tile_base, seg = _layout(tiles)
    S = TT * BLK

    order = np.lexsort((local, block, par, sbk, core))
    s_s = s_all[order]
    d_s = d_all[order]
    core_s = core[order]
    block_s = block[order]
    h_s = par[order]

    gid = (core_s * NBLK + block_s) * 2 + h_s
    change = np.r_[True, gid[1:] != gid[:-1]]
    gstart = np.maximum.accumulate(np.where(change, np.arange(len(gid)), 0))
    rank = np.arange(len(gid)) - gstart
    slot = tile_base[block_s, h_s] * BLK + rank  # per-core slot in [0, S)

    src_loc = (_newpos(s_s & ~1) >> 1).astype(np.int16)  # permuted pair idx
    dst_loc = (d_s % NSH - block_s * BLK).astype(np.float32)  # 0..127
    norm = dinv[d_s] * dinv[s_s]

    seg_slot0_tbl = np.zeros(TT, dtype=np.int64)  # per tile: slot0 of its seg
    for _key, (t0, nt) in seg.items():
        seg_slot0_tbl[t0 : t0 + nt] = t0 * BLK

    meta_np = np.zeros((NCORES, 128, TT, 2), dtype=np.float32)
    idx_np = np.zeros((NCORES, 128, S // 16), dtype=np.int16)
    for c in range(NCORES):
        m = core_s == c
        sl = slot[m]
        tt = sl // BLK
        pp = sl % BLK
        meta_np[c, pp, tt, 0] = dst_loc[m]
        meta_np[c, pp, tt, 1] = norm[m]
        seg0 = seg_slot0_tbl[tt]
        j = sl - seg0
        col = seg0 // 16 + j // 16
        row = j % 16
        v = src_loc[m]
        for g in range(8):  # replicate across the 8 gpsimd 16-partition groups
            idx_np[c, row + 16 * g, col] = v
    allpos = _newpos(np.arange(N, dtype=np.int64))
    order_nodes = np.empty(N, dtype=np.int64)
    order_nodes[allpos] = np.arange(N, dtype=np.int64)
    return tiles, TT, tile_base, seg, meta_np, idx_np, dinv, order_nodes


def _build(prep):
    tiles, TT, tile_base, seg, _meta, _idx, _dinv, _ord = prep
    S = TT * BLK
    f32 = mybir.dt.float32
    bf16 = mybir.dt.bfloat16
    i16 = mybir.dt.int16
    AF = mybir.ActivationFunctionType
    OP = mybir.AluOpType

    nc = bacc.Bacc(
        "TRN2",
        target_bir_lowering=False,
        debug=False,
        num_devices=NCORES,
        num_swdge_queues=NQ,
    )
    xrow = nc.dram_tensor("xrow", [N, FIN], bf16, kind="ExternalInput")
    w1 = nc.dram_tensor("w1", [128, HID], bf16, kind="ExternalInput")
    w2 = nc.dram_tensor("w2", [128, FOUT], bf16, kind="ExternalInput")
    b1c = nc.dram_tensor("b1c", [128, 1], f32, kind="ExternalInput")
    b2r = nc.dram_tensor("b2r", [128, FOUT], f32, kind="ExternalInput")
    iot = nc.dram_tensor("iot", [128, BLK], bf16, kind="ExternalInput")
    meta = nc.dram_tensor("meta", [128, TT, 2], f32, kind="ExternalInput")
    idxt = nc.dram_tensor("idx", [128, S // 16], i16, kind="ExternalInput")
    xself = nc.dram_tensor("xself", [128, NSHP], bf16, kind="ExternalInput")
    ident = nc.dram_tensor("ident", [128, BLK], bf16, kind="ExternalInput")
    diag2 = nc.dram_tensor("diag2", [128, NBLK, BLK], bf16, kind="ExternalInput")
    outp = nc.dram_tensor("out", [NSH, FOUT], f32, kind="ExternalOutput")

    with tile.TileContext(nc) as tc, ExitStack() as ctx:
        const = ctx.enter_context(tc.tile_pool(name="const", bufs=1))
        dram = ctx.enter_context(tc.tile_pool(name="dram", bufs=1, space="DRAM"))
        gp1 = ctx.enter_context(tc.tile_pool(name="g1", bufs=2))
        gp2 = ctx.enter_context(tc.tile_pool(name="g2", bufs=2))
        tpp = ctx.enter_context(tc.tile_pool(name="tp", bufs=40))
        evp = ctx.enter_context(tc.tile_pool(name="ev", bufs=4))
        psa = ctx.enter_context(tc.tile_pool(name="psa", bufs=3, space="PSUM"))
        psb = ctx.enter_context(tc.tile_pool(name="psb", bufs=1, space="PSUM"))
        psc = ctx.enter_context(tc.tile_pool(name="psc", bufs=1, space="PSUM"))
        psd = ctx.enter_context(tc.tile_pool(name="psd", bufs=3, space="PSUM"))

        def cload(ap, shape, dtype, tag):
            t = const.tile(shape, dtype, tag=tag)
            nc.sync.dma_start(t[:], ap)
            return t

        idx_sb = cload(idxt[:, :], [128, S // 16], i16, "idx")
        iota_sb = cload(iot[:, :], [128, BLK], bf16, "iota")
        meta_sb = cload(meta[:, :, :], [128, TT, 2], f32, "meta")
        w1_sb = cload(w1[:, :], [128, HID], bf16, "w1")
        w2_sb = cload(w2[:, :], [128, FOUT], bf16, "w2")
        b1_sb = cload(b1c[:, :], [128, 1], f32, "b1")
        b2_sb = cload(b2r[:, :], [128, FOUT], f32, "b2")
        xself_sb = cload(xself[:, :], [128, NSHP], bf16, "xself")
        ident_sb = cload(ident[:, :], [128, BLK], bf16, "ident")
        diag_sb = cload(diag2[:, :, :], [128, NBLK, BLK], bf16, "diag2")

        t2loc = dram.tile([NSHP, FOUT], bf16, tag="t2loc")
        t2full = dram.tile([N, FOUT], bf16, tag="t2full")
        # pair view for the layer-2 gather: row p = nodes (2p, 2p+1), 256B
        t2pair = t2full[:, :].rearrange("(p two) f -> p (two f)", two=2)
        # even/odd views of X for the layer-1 gather (elem_step = 2 rows)
        xpar = xrow[:, :].rearrange("(p two) f -> p two f", two=2)

        # zero-fill the padded t2loc tail (layer-2 diag matmul reads it)
        zt = evp.tile([128, FOUT], bf16, tag="zt")
        nc.vector.memset(zt[:], 0)
        nc.sync.dma_start(t2loc[NSH:NSHP, :], zt[0 : NSHP - NSH, :])

        qctr = [0]

        def gather(g, view, t0, ntl, elem, elem_step=None):
            for q0 in range(0, ntl, GCH):
                qn = min(GCH, ntl - q0)
                c0 = (t0 + q0) * 8  # idx columns (tile*128/16)
                nc.gpsimd.dma_gather(
                    out_ap=g[:, q0 : q0 + qn, :],
                    in_ap=view,
                    idxs_ap=idx_sb[:, c0 : c0 + qn * 8],
                    num_idxs=qn * 128,
                    num_idxs_reg=qn * 128,
                    elem_size=elem,
                    elem_step=elem_step,
                    queue_num=qctr[0] % NQ,
                )
                qctr[0] += 1

        def onehot(t):
            tp = tpp.tile([128, BLK], bf16, tag="tp")
            nc.vector.tensor_scalar(
                out=tp[:],
                in0=iota_sb[:],
                scalar1=meta_sb[:, t, 0:1],
                scalar2=meta_sb[:, t, 1:2],
                op0=OP.is_equal,
                op1=OP.mult,
            )
            return tp

        # ---- Layer 1: aggregate X, project W1 (+b1, relu) and W2 per block
        for sb in range(NSB):
            blocks = list(range(sb * SBW, min((sb + 1) * SBW, NBLK)))
            gt = {}
            for h in (0, 1):
                t0, ntl = seg[(sb, h)]
                if ntl == 0:
                    continue
                g = gp1.tile([128, ntl, FIN], bf16, tag=f"g1{h}")
                if ABLATE >= 1:
                    gather(g, xpar[:, h, :], t0, ntl, FIN, elem_step=2 * FIN)
                else:
                    nc.vector.memset(g[:], 0)
                gt[h] = g
            for b in blocks:
                r0 = b * BLK
                r1 = min(NSH, r0 + BLK)
                ps2 = psa.tile([128, BLK], f32, tag="psa")  # [xfeat, dst]
                # self-loop diagonal: identity lhsT copies the prescaled block
                nc.tensor.matmul(
                    ps2[:],
                    lhsT=ident_sb[:],
                    rhs=xself_sb[:, r0 : r0 + BLK],
                    start=True,
                    stop=False,
                )
                nmm = sum(int(tiles[b][h]) for h in (0, 1))
                done = 0
                for h in (0, 1):
                    seg_t0 = seg[(sb, h)][0]
                    for k in range(int(tiles[b][h])):
                        t = int(tile_base[b][h]) + k
                        gofs = t - seg_t0
                        tp = onehot(t)
                        done += 1
                        nc.tensor.matmul(
                            ps2[:],
                            lhsT=gt[h][:, gofs, :],
                            rhs=tp[:],
                            start=False,
                            stop=done == nmm,
                        )
                a2 = evp.tile([128, BLK], bf16, tag="a2")
                nc.scalar.activation(a2[:], ps2[:], AF.Copy)
                poT = psb.tile([128, BLK], f32, tag="psb")  # [hid, dst]
                nc.tensor.matmul(
                    poT[:], lhsT=w1_sb[:], rhs=a2[:], start=True, stop=True
                )
                hbT = evp.tile([128, BLK], bf16, tag="hbT")
                nc.scalar.activation(hbT[:], poT[:], AF.Relu, bias=b1_sb[:, 0:1])
                t2w = psc.tile([128, FOUT], f32, tag="psc")  # [dst, fout]
                nc.tensor.matmul(
                    t2w[:], lhsT=hbT[:], rhs=w2_sb[:], start=True, stop=True
                )
                t2e = evp.tile([128, FOUT], bf16, tag="t2e")
                nc.scalar.activation(t2e[:], t2w[:], AF.Copy)
                nc.sync.dma_start(t2loc[r0:r1, :], t2e[0 : r1 - r0, :])

        # ---- share the 64-wide t2 table (chunked to overlap layer 1);
        # t2full rows are in (chunk, core, row) order so each chunk's
        # output is one contiguous slice.
        for q, (r0, r1) in enumerate(CCHUNKS):
            g0 = int(_CHB[q])
            g1 = int(_CHB[q + 1])
            if ABLATE >= 2:
                nc.gpsimd.collective_compute(
                    "AllGather",
                    mybir.AluOpType.bypass,
                    replica_groups=[list(range(NCORES))],
                    ins=[t2loc[r0:r1, :].opt()],
                    outs=[t2full[g0:g1, :].opt()],
                )
            else:
                nc.sync.dma_start(t2full[g0 : g0 + (r1 - r0), :], t2loc[r0:r1, :])

        # ---- Layer 2: aggregate t2 pair rows (self diag via dense block), +b2
        for sb in range(NSB):
            blocks = list(range(sb * SBW, min((sb + 1) * SBW, NBLK)))
            t0sb = seg[(sb, 0)][0]
            ntlsb = seg[(sb, 0)][1] + seg[(sb, 1)][1]
            g = gp2.tile([128, ntlsb, 2 * FOUT], bf16, tag="g2")
            if ABLATE >= 1:
                gather(g, t2pair, t0sb, ntlsb, 2 * FOUT)
            else:
                nc.vector.memset(g[:], 0)
            gt = {0: g, 1: g}
            for b in blocks:
                r0 = b * BLK
                r1 = min(NSH, r0 + BLK)
                t2s = evp.tile([128, FOUT], bf16, tag="t2s")
                nc.sync.dma_start(t2s[:], t2loc[r0 : r0 + BLK, :])
                ps = psd.tile([128, FOUT], f32, tag="psd")  # [dst, fout]
                nc.tensor.matmul(
                    ps[:], lhsT=diag_sb[:, b, :], rhs=t2s[:], start=True, stop=False
                )
                nmm = sum(int(tiles[b][h]) for h in (0, 1))
                done = 0
                for h in (0, 1):
                    for k in range(int(tiles[b][h])):
                        t = int(tile_base[b][h]) + k
                        gofs = t - t0sb
                        tp = onehot(t)
                        done += 1
                        nc.tensor.matmul(
                            ps[:],
                            lhsT=tp[:],
                            rhs=gt[h][:, gofs, h * FOUT : (h + 1) * FOUT],
                            start=False,
                            stop=done == nmm,
                        )
                oo = evp.tile([128, FOUT], f32, tag="oo")
                nc.vector.tensor_add(oo[:], ps[:], b2_sb[:])
                nc.sync.dma_start(outp[r0:r1, :], oo[0 : r1 - r0, :])

    nc.finalize()
    return nc


def _in_maps(x, W1, b1, W2, b2, prep):
    tiles, TT, tile_base, seg, meta_np, idx_np, dinv, order_nodes = prep
    xf = np.asarray(x, np.float32)
    xr = np.ascontiguousarray(xf[order_nodes].astype(BF16))
    w1b = np.asarray(W1, np.float32).astype(BF16)
    w2b = np.asarray(W2, np.float32).astype(BF16)
    b1col = np.asarray(b1, np.float32).reshape(128, 1).copy()
    b2rep = np.broadcast_to(np.asarray(b2, np.float32), (128, FOUT)).copy()
    iota = np.broadcast_to(np.arange(BLK, dtype=np.float32), (128, BLK)).astype(BF16)
    shared = {
        "xrow": xr,
        "w1": w1b,
        "w2": w2b,
        "b1c": b1col,
        "b2r": b2rep,
        "iot": np.ascontiguousarray(iota),
        "ident": np.ascontiguousarray(np.eye(BLK, dtype=np.float32).astype(BF16)),
    }
    d2 = (dinv.astype(np.float64) ** 2).astype(np.float32)
    maps = []
    for c in range(NCORES):
        lo, hi = c * NSH, (c + 1) * NSH
        xs = np.zeros((128, NSHP), dtype=BF16)
        xs[:, :NSH] = (xf[lo:hi] * d2[lo:hi, None]).T.astype(BF16)
        dvals = np.zeros(NSHP, np.float32)
        dvals[:NSH] = d2[lo:hi]
        dg = np.zeros((128, NBLK, BLK), dtype=np.float32)
        for b in range(NBLK):
            dg[np.arange(BLK), b, np.arange(BLK)] = dvals[b * BLK : (b + 1) * BLK]
        maps.append(
            dict(
                shared,
                meta=np.ascontiguousarray(meta_np[c]),
                idx=np.ascontiguousarray(idx_np[c]),
                xself=xs,
                diag2=np.ascontiguousarray(dg.astype(BF16)),
            )
        )
    return maps


def kernel(x, edge_index, W1, b1, W2, b2):
    prep = _prep(edge_index)
    nc = _build(prep)
    in_maps = _in_maps(x, W1, b1, W2, b2, prep)
    res = run_bass_kernel_spmd(nc, in_maps, core_ids=list(range(NCORES)), trace=False)
    out = np.concatenate(
        [res.results[c]["out"].astype(np.float32) for c in range(NCORES)], axis=0
    )
    return out


# revision 15
# speedup vs baseline: 1.2972x; 1.1934x over previous
"""Two-layer GCN (PyG GCNConv semantics) on 8 Trainium2 NeuronCores.

Math: out = Ahat @ relu(Ahat @ (X@W1) + b1) @ W2 + b2, with
Ahat = D^-1/2 (A + I) D^-1/2.  Edge norm dinv[src]*dinv[dst] is folded
into per-edge one-hot aggregation matrices (segmented matmul on the PE,
edges sorted by dst, 128-edge tiles).  Self-loops are NOT materialized
as edges: their diagonal contribution dinv[d]^2 * row[d] is added per
128-dst block with one dense matmul against host-prescaled tensors.

Layer 1 defers W1: aggregate raw X rows (gathered by edge src via gpsimd
dma_gather), then per 128-dst block project the aggregate through W1
(+b1, relu) AND W2, storing t2 = relu(.)@W2 (64 wide, bf16).

Both layers split edges by src PARITY, giving one shared tile layout:
the int16 gather index is src>>1 for both (fits 25000 < 32768).
Layer 1 gathers 256B X rows through an even/odd strided view
(elem_step=256 elems = 2 rows); layer 2 gathers 256B PAIR rows of the
64-wide t2 table (the parity picks the rhs column half).

Sharding: destination nodes split across 8 cores (6250 each); one
AllGather shares the 64-wide layer-2 source table.
"""

import sys

import numpy as np

try:
    import concourse.bass as bass  # noqa: F401
except ImportError:
    sys.path.insert(0, "/opt/trn_rl_repo")

from contextlib import ExitStack

import ml_dtypes

import concourse.bass as bass
import concourse.tile as tile
from concourse import bacc, mybir
from concourse.bass_utils import run_bass_kernel_spmd

BF16 = ml_dtypes.bfloat16

# debug ablation: 0 = no dma_gather + no collective, 1 = gather + no collective,
# 2 = full kernel
ABLATE = 2

N = 50000
E = 800000
FIN = 128
HID = 128
FOUT = 64
NCORES = 8
NSH = N // NCORES  # 6250 destination nodes per core
BLK = 128  # dst block (psum window)
NBLK = (NSH + BLK - 1) // BLK  # 49
NSHP = NBLK * BLK  # 6272, padded
SBW = 4  # dst blocks per superblock
NSB = (NBLK + SBW - 1) // SBW  # 13
GCH = 8  # gather chunk: 8 tiles = 1024 idx (SWDGE ring limit, ucode-fixed)
NQ = 4  # swdge queues, round-robined across gather calls
# collective chunks (row ranges of t2loc), aligned to superblock boundaries
CCHUNKS = [(0, 1536), (1536, 3072), (3072, 4096), (4096, 5120), (5120, 5632), (5632, NSH)]
# t2full/xrow row permutation: (chunk, core, row) order makes each chunk's
# AllGather output a contiguous slice.  newpos preserves row parity.
_CHB = np.cumsum([0] + [NCORES * (r1 - r0) for r0, r1 in CCHUNKS])


def _newpos(n):
    c, local = n // NSH, n % NSH
    pos = np.zeros_like(n)
    for q, (r0, r1) in enumerate(CCHUNKS):
        m = (local >= r0) & (local < r1)
        pos[m] = _CHB[q] + c[m] * (r1 - r0) + (local[m] - r0)
    return pos


def _layout(tiles):
    """Static program layout from per-(block, parity) tile counts.

    Data order: (sb, parity, block); gather segments keyed (sb, parity).
    Returns (TT, tile_base[NBLK][2], seg: (sb, par) -> (tile0, ntiles)).
    """
    tile_base = np.zeros((NBLK, 2), dtype=np.int64)
    seg = {}
    pos = 0
    for sb in range(NSB):
        blocks = range(sb * SBW, min((sb + 1) * SBW, NBLK))
        for h in (0, 1):
            seg_start = pos
            for b in blocks:
                tile_base[b][h] = pos
                pos += int(tiles[b][h])
            seg[(sb, h)] = (seg_start, pos - seg_start)
    return int(pos), tile_base, seg


def _prep(edge_index):
    """Shared (both layers) tile layout + per-core meta/idx.

    Returns (tiles, TT, tile_base, seg, meta, idx, dinv).
    """
    s_all = np.asarray(edge_index[0], dtype=np.int64)
    d_all = np.asarray(edge_index[1], dtype=np.int64)
    deg = (np.bincount(d_all, minlength=N) + 1).astype(np.float64)
    dinv = (1.0 / np.sqrt(deg)).astype(np.float32)

    core = d_all // NSH
    local = d_all % NSH
    block = local // BLK
    sbk = block // SBW
    par = (s_all & 1).astype(np.int64)

    cidx = (core * NBLK + block) * 2 + par
    cnt = np.bincount(cidx, minlength=NCORES * NBLK * 2).reshape(NCORES, NBLK, 2)
    tiles = ((cnt + BLK - 1) // BLK).max(axis=0)  # [NBLK, 2] max over cores

    TT, tile_base, seg = _layout(tiles)
    S = TT * BLK

    order = np.lexsort((local, block, par, sbk, core))
    s_s = s_all[order]
    d_s = d_all[order]
    core_s = core[order]
    block_s = block[order]
    h_s = par[order]

    gid = (core_s * NBLK + block_s) * 2 + h_s
    change = np.r_[True, gid[1:] != gid[:-1]]
    gstart = np.maximum.accumulate(np.where(change, np.arange(len(gid)), 0))
    rank = np.arange(len(gid)) - gstart
    slot = tile_base[block_s, h_s] * BLK + rank  # per-core slot in [0, S)

    src_loc = (_newpos(s_s & ~1) >> 1).astype(np.int16)  # permuted pair idx
    dst_loc = (d_s % NSH - block_s * BLK).astype(np.float32)  # 0..127
    norm = dinv[d_s] * dinv[s_s]

    seg_slot0_tbl = np.zeros(TT, dtype=np.int64)  # per tile: slot0 of its seg
    for _key, (t0, nt) in seg.items():
        seg_slot0_tbl[t0 : t0 + nt] = t0 * BLK

    meta_np = np.zeros((NCORES, 128, TT, 2), dtype=np.float32)
    idx_np = np.zeros((NCORES, 128, S // 16), dtype=np.int16)
    for c in range(NCORES):
        m = core_s == c
        sl = slot[m]
        tt = sl // BLK
        pp = sl % BLK
        meta_np[c, pp, tt, 0] = dst_loc[m]
        meta_np[c, pp, tt, 1] = norm[m]
        seg0 = seg_slot0_tbl[tt]
        j = sl - seg0
        col = seg0 // 16 + j // 16
        row = j % 16
        v = src_loc[m]
        for g in range(8):  # replicate across the 8 gpsimd 16-partition groups
            idx_np[c, row + 16 * g, col] = v
    allpos = _newpos(np.arange(N, dtype=np.int64))
    order_nodes = np.empty(N, dtype=np.int64)
    order_nodes[allpos] = np.arange(N, dtype=np.int64)
    return tiles, TT, tile_base, seg, meta_np, idx_np, dinv, order_nodes


def _build(prep):
    tiles, TT, tile_base, seg, _meta, _idx, _dinv, _ord = prep
    S = TT * BLK
    f32 = mybir.dt.float32
    bf16 = mybir.dt.bfloat16
    i16 = mybir.dt.int16
    AF = mybir.ActivationFunctionType
    OP = mybir.AluOpType

    nc = bacc.Bacc(
        "TRN2",
        target_bir_lowering=False,
        debug=False,
        num_devices=NCORES,
        num_swdge_queues=NQ,
    )
    xrow = nc.dram_tensor("xrow", [N, FIN], bf16, kind="ExternalInput")
    w1 = nc.dram_tensor("w1", [128, HID], bf16, kind="ExternalInput")
    w2 = nc.dram_tensor("w2", [128, FOUT], bf16, kind="ExternalInput")
    b1c = nc.dram_tensor("b1c", [128, 1], f32, kind="ExternalInput")
    b2r = nc.dram_tensor("b2r", [128, FOUT], f32, kind="ExternalInput")
    iot = nc.dram_tensor("iot", [128, BLK], bf16, kind="ExternalInput")
    meta = nc.dram_tensor("meta", [128, TT, 2], f32, kind="ExternalInput")
    idxt = nc.dram_tensor("idx", [128, S // 16], i16, kind="ExternalInput")
    xself = nc.dram_tensor("xself", [128, NSHP], bf16, kind="ExternalInput")
    ident = nc.dram_tensor("ident", [128, BLK], bf16, kind="ExternalInput")
    diag2 = nc.dram_tensor("diag2", [128, NBLK, BLK], bf16, kind="ExternalInput")
    outp = nc.dram_tensor("out", [NSH, FOUT], f32, kind="ExternalOutput")

    with tile.TileContext(nc) as tc, ExitStack() as ctx:
        const = ctx.enter_context(tc.tile_pool(name="const", bufs=1))
        dram = ctx.enter_context(tc.tile_pool(name="dram", bufs=1, space="DRAM"))
        gp1 = ctx.enter_context(tc.tile_pool(name="g1", bufs=2))
        gp2 = ctx.enter_context(tc.tile_pool(name="g2", bufs=2))
        tpp = ctx.enter_context(tc.tile_pool(name="tp", bufs=40))
        evp = ctx.enter_context(tc.tile_pool(name="ev", bufs=4))
        psa = ctx.enter_context(tc.tile_pool(name="psa", bufs=3, space="PSUM"))
        psb = ctx.enter_context(tc.tile_pool(name="psb", bufs=1, space="PSUM"))
        psc = ctx.enter_context(tc.tile_pool(name="psc", bufs=1, space="PSUM"))
        psd = ctx.enter_context(tc.tile_pool(name="psd", bufs=3, space="PSUM"))

        def cload(ap, shape, dtype, tag):
            t = const.tile(shape, dtype, tag=tag)
            nc.sync.dma_start(t[:], ap)
            return t

        idx_sb = cload(idxt[:, :], [128, S // 16], i16, "idx")
        iota_sb = cload(iot[:, :], [128, BLK], bf16, "iota")
        meta_sb = cload(meta[:, :, :], [128, TT, 2], f32, "meta")
        w1_sb = cload(w1[:, :], [128, HID], bf16, "w1")
        w2_sb = cload(w2[:, :], [128, FOUT], bf16, "w2")
        b1_sb = cload(b1c[:, :], [128, 1], f32, "b1")
        b2_sb = cload(b2r[:, :], [128, FOUT], f32, "b2")
        xself_sb = cload(xself[:, :], [128, NSHP], bf16, "xself")
        ident_sb = cload(ident[:, :], [128, BLK], bf16, "ident")
        diag_sb = cload(diag2[:, :, :], [128, NBLK, BLK], bf16, "diag2")

        t2loc = dram.tile([NSHP, FOUT], bf16, tag="t2loc")
        t2full = dram.tile([N, FOUT], bf16, tag="t2full")
        # pair view for the layer-2 gather: row p = nodes (2p, 2p+1), 256B
        t2pair = t2full[:, :].rearrange("(p two) f -> p (two f)", two=2)
        # even/odd views of X for the layer-1 gather (elem_step = 2 rows)
        xpar = xrow[:, :].rearrange("(p two) f -> p two f", two=2)

        # zero-fill the padded t2loc tail (layer-2 diag matmul reads it)
        zt = evp.tile([128, FOUT], bf16, tag="zt")
        nc.vector.memset(zt[:], 0)
        nc.sync.dma_start(t2loc[NSH:NSHP, :], zt[0 : NSHP - NSH, :])

        qctr = [0]

        def gather(g, view, t0, ntl, elem, elem_step=None):
            for q0 in range(0, ntl, GCH):
                qn = min(GCH, ntl - q0)
                c0 = (t0 + q0) * 8  # idx columns (tile*128/16)
                nc.gpsimd.dma_gather(
                    out_ap=g[:, q0 : q0 + qn, :],
                    in_ap=view,
                    idxs_ap=idx_sb[:, c0 : c0 + qn * 8],
                    num_idxs=qn * 128,
                    num_idxs_reg=qn * 128,
                    elem_size=elem,
                    elem_step=elem_step,
                    queue_num=qctr[0] % NQ,
                )
                qctr[0] += 1

        def onehot(t):
            tp = tpp.tile([128, BLK], bf16, tag="tp")
            nc.vector.tensor_scalar(
                out=tp[:],
                in0=iota_sb[:],
                scalar1=meta_sb[:, t, 0:1],
                scalar2=meta_sb[:, t, 1:2],
                op0=OP.is_equal,
                op1=OP.mult,
            )
            return tp

        # ---- Layer 1: aggregate X, project W1 (+b1, relu) and W2 per block
        for sb in range(NSB):
            blocks = list(range(sb * SBW, min((sb + 1) * SBW, NBLK)))
            gt = {}
            for h in (0, 1):
                t0, ntl = seg[(sb, h)]
                if ntl == 0:
                    continue
                g = gp1.tile([128, ntl, FIN], bf16, tag=f"g1{h}")
                if ABLATE >= 1:
                    gather(g, xpar[:, h, :], t0, ntl, FIN, elem_step=2 * FIN)
                else:
                    nc.vector.memset(g[:], 0)
                gt[h] = g
            for b in blocks:
                r0 = b * BLK
                r1 = min(NSH, r0 + BLK)
                ps2 = psa.tile([128, BLK], f32, tag="psa")  # [xfeat, dst]
                # self-loop diagonal: identity lhsT copies the prescaled block
                nc.tensor.matmul(
                    ps2[:],
                    lhsT=ident_sb[:],
                    rhs=xself_sb[:, r0 : r0 + BLK],
                    start=True,
                    stop=False,
                )
                nmm = sum(int(tiles[b][h]) for h in (0, 1))
                done = 0
                for h in (0, 1):
                    seg_t0 = seg[(sb, h)][0]
                    for k in range(int(tiles[b][h])):
                        t = int(tile_base[b][h]) + k
                        gofs = t - seg_t0
                        tp = onehot(t)
                        done += 1
                        nc.tensor.matmul(
                            ps2[:],
                            lhsT=gt[h][:, gofs, :],
                            rhs=tp[:],
                            start=False,
                            stop=done == nmm,
                        )
                a2 = evp.tile([128, BLK], bf16, tag="a2")
                nc.scalar.activation(a2[:], ps2[:], AF.Copy)
                poT = psb.tile([128, BLK], f32, tag="psb")  # [hid, dst]
                nc.tensor.matmul(
                    poT[:], lhsT=w1_sb[:], rhs=a2[:], start=True, stop=True
                )
                hbT = evp.tile([128, BLK], bf16, tag="hbT")
                nc.scalar.activation(hbT[:], poT[:], AF.Relu, bias=b1_sb[:, 0:1])
                t2w = psc.tile([128, FOUT], f32, tag="psc")  # [dst, fout]
                nc.tensor.matmul(
                    t2w[:], lhsT=hbT[:], rhs=w2_sb[:], start=True, stop=True
                )
                t2e = evp.tile([128, FOUT], bf16, tag="t2e")
                nc.scalar.activation(t2e[:], t2w[:], AF.Copy)
                nc.sync.dma_start(t2loc[r0:r1, :], t2e[0 : r1 - r0, :])

        # ---- share the 64-wide t2 table (chunked to overlap layer 1);
        # t2full rows are in (chunk, core, row) order so each chunk's
        # output is one contiguous slice.
        for q, (r0, r1) in enumerate(CCHUNKS):
            g0 = int(_CHB[q])
            g1 = int(_CHB[q + 1])
            if ABLATE >= 2:
                nc.gpsimd.collective_compute(
                    "AllGather",
                    mybir.AluOpType.bypass,
                    replica_groups=[list(range(NCORES))],
                    ins=[t2loc[r0:r1, :].opt()],
                    outs=[t2full[g0:g1, :].opt()],
                )
            else:
                nc.sync.dma_start(t2full[g0 : g0 + (r1 - r0), :], t2loc[r0:r1, :])

        # ---- Layer 2: aggregate t2 pair rows (self diag via dense block), +b2
        for sb in range(NSB):
            blocks = list(range(sb * SBW, min((sb + 1) * SBW, NBLK)))
            t0sb = seg[(sb, 0)][0]
            ntlsb = seg[(sb, 0)][1] + seg[(sb, 1)][1]
            g = gp2.tile([128, ntlsb, 2 * FOUT], bf16, tag="g2")
            if ABLATE >= 1:
                gather(g, t2pair, t0sb, ntlsb, 2 * FOUT)
            else:
                nc.vector.memset(g[:], 0)
            gt = {0: g, 1: g}
            for b in blocks:
                r0 = b * BLK
                r1 = min(NSH, r0 + BLK)
                t2s = evp.tile([128, FOUT], bf16, tag="t2s")
                nc.sync.dma_start(t2s[:], t2loc[r0 : r0 + BLK, :])
                ps = psd.tile([128, FOUT], f32, tag="psd")  # [dst, fout]
                nc.tensor.matmul(
                    ps[:], lhsT=diag_sb[:, b, :], rhs=t2s[:], start=True, stop=False
                )
                nmm = sum(int(tiles[b][h]) for h in (0, 1))
                done = 0
                for h in (0, 1):
                    for k in range(int(tiles[b][h])):
                        t = int(tile_base[b][h]) + k
                        gofs = t - t0sb
                        tp = onehot(t)
                        done += 1
                        nc.tensor.matmul(
                            ps[:],
                            lhsT=tp[:],
                            rhs=gt[h][:, gofs, h * FOUT : (h + 1) * FOUT],
                            start=False,
                            stop=done == nmm,
                        )
                oo = evp.tile([128, FOUT], f32, tag="oo")
                nc.vector.tensor_add(oo[:], ps[:], b2_sb[:])
                nc.sync.dma_start(outp[r0:r1, :], oo[0 : r1 - r0, :])

    nc.finalize()
    return nc


def _in_maps(x, W1, b1, W2, b2, prep):
    tiles, TT, tile_base, seg, meta_np, idx_np, dinv, order_nodes = prep
    xf = np.asarray(x, np.float32)
    xr = np.ascontiguousarray(xf[order_nodes].astype(BF16))
    w1b = np.asarray(W1, np.float32).astype(BF16)
    w2b = np.asarray(W2, np.float32).astype(BF16)
    b1col = np.asarray(b1, np.float32).reshape(128, 1).copy()
    b2rep = np.broadcast_to(np.asarray(b2, np.float32), (128, FOUT)).copy()
    iota = np.broadcast_to(np.arange(BLK, dtype=np.float32), (128, BLK)).astype(BF16)
    shared = {
        "xrow": xr,
        "w1": w1b,
        "w2": w2b,
        "b1c": b1col,
        "b2r": b2rep,
        "iot": np.ascontiguousarray(iota),
        "ident": np.ascontiguousarray(np.eye(BLK, dtype=np.float32).astype(BF16)),
    }
    d2 = (dinv.astype(np.float64) ** 2).astype(np.float32)
    maps = []
    for c in range(NCORES):
        lo, hi = c * NSH, (c + 1) * NSH
        xs = np.zeros((128, NSHP), dtype=BF16)
        xs[:, :NSH] = (xf[lo:hi] * d2[lo:hi, None]).T.astype(BF16)
        dvals = np.zeros(NSHP, np.float32)
        dvals[:NSH] = d2[lo:hi]
        dg = np.zeros((128, NBLK, BLK), dtype=np.float32)
        for b in range(NBLK):
            dg[np.arange(BLK), b, np.arange(BLK)] = dvals[b * BLK : (b + 1) * BLK]
        maps.append(
            dict(
                shared,
                meta=np.ascontiguousarray(meta_np[c]),
                idx=np.ascontiguousarray(idx_np[c]),
                xself=xs,
                diag2=np.ascontiguousarray(dg.astype(BF16)),
            )
        )
    return maps


def kernel(x, edge_index, W1, b1, W2, b2):
    prep = _prep(edge_index)
    nc = _build(prep)
    in_maps = _in_maps(x, W1, b1, W2, b2, prep)
    res = run_bass_kernel_spmd(nc, in_maps, core_ids=list(range(NCORES)), trace=False)
    out = np.concatenate(
        [res.results[c]["out"].astype(np.float32) for c in range(NCORES)], axis=0
    )
    return out


# revision 17
# speedup vs baseline: 1.4450x; 1.1140x over previous
"""Two-layer GCN (PyG GCNConv semantics) on 8 Trainium2 NeuronCores.

Math: out = Ahat @ relu(Ahat @ (X@W1) + b1) @ W2 + b2, with
Ahat = D^-1/2 (A + I) D^-1/2.  Edge norm dinv[src]*dinv[dst] is folded
into per-edge one-hot aggregation matrices (segmented matmul on the PE,
edges sorted by dst, 128-edge tiles).  Self-loops are NOT materialized
as edges: their diagonal contribution dinv[d]^2 * row[d] is added per
128-dst block with one dense matmul against host-prescaled tensors.

Layer 1 defers W1: aggregate raw X rows (gathered by edge src via gpsimd
dma_gather), then per 128-dst block project the aggregate through W1
(+b1, relu) AND W2, storing t2 = relu(.)@W2 (64 wide, bf16).

Both layers split edges by src PARITY, giving one shared tile layout:
the int16 gather index is src>>1 for both (fits 25000 < 32768).
Layer 1 gathers 256B X rows through an even/odd strided view
(elem_step=256 elems = 2 rows); layer 2 gathers 256B PAIR rows of the
64-wide t2 table (the parity picks the rhs column half).

Sharding: destination nodes split across 8 cores (6250 each); one
AllGather shares the 64-wide layer-2 source table.
"""

import sys

import numpy as np

try:
    import concourse.bass as bass  # noqa: F401
except ImportError:
    sys.path.insert(0, "/opt/trn_rl_repo")

from contextlib import ExitStack

import ml_dtypes

import concourse.bass as bass
import concourse.tile as tile
from concourse import bacc, mybir
from concourse.bass_utils import run_bass_kernel_spmd

BF16 = ml_dtypes.bfloat16

# debug ablation: 0 = no dma_gather + no collective, 1 = gather + no collective,
# 2 = full kernel
ABLATE = 2

N = 50000
E = 800000
FIN = 128
HID = 128
FOUT = 64
NCORES = 8
NSH = N // NCORES  # 6250 destination nodes per core
BLK = 128  # dst block (psum window)
NBLK = (NSH + BLK - 1) // BLK  # 49
NSHP = NBLK * BLK  # 6272, padded
SBW = 4  # dst blocks per superblock
NSB = (NBLK + SBW - 1) // SBW  # 13
GCH = 8  # gather chunk: 8 tiles = 1024 idx (SWDGE ring limit, ucode-fixed)
NQ = 4  # swdge queues, round-robined across gather calls
# collective chunks (row ranges of t2loc), aligned to superblock boundaries
CCHUNKS = [(0, 1536), (1536, 3072), (3072, 4096), (4096, 5120), (5120, 5632), (5632, NSH)]
# t2full/xrow row permutation: (chunk, core, row) order makes each chunk's
# AllGather output a contiguous slice.  newpos preserves row parity.
_CHB = np.cumsum([0] + [NCORES * (r1 - r0) for r0, r1 in CCHUNKS])


def _newpos(n):
    c, local = n // NSH, n % NSH
    pos = np.zeros_like(n)
    for q, (r0, r1) in enumerate(CCHUNKS):
        m = (local >= r0) & (local < r1)
        pos[m] = _CHB[q] + c[m] * (r1 - r0) + (local[m] - r0)
    return pos


def _layout(tiles):
    """Static program layout from per-(block, parity) tile counts.

    Data order: (sb, parity, block); gather segments keyed (sb, parity).
    Returns (TT, tile_base[NBLK][2], seg: (sb, par) -> (tile0, ntiles)).
    """
    tile_base = np.zeros((NBLK, 2), dtype=np.int64)
    seg = {}
    pos = 0
    for sb in range(NSB):
        blocks = range(sb * SBW, min((sb + 1) * SBW, NBLK))
        for h in (0, 1):
            seg_start = pos
            for b in blocks:
                tile_base[b][h] = pos
                pos += int(tiles[b][h])
            seg[(sb, h)] = (seg_start, pos - seg_start)
    return int(pos), tile_base, seg


def _prep(edge_index):
    """Shared (both layers) tile layout + per-core meta/idx.

    Returns (tiles, TT, tile_base, seg, meta, idx, dinv).
    """
    s_all = np.asarray(edge_index[0], dtype=np.int64)
    d_all = np.asarray(edge_index[1], dtype=np.int64)
    deg = (np.bincount(d_all, minlength=N) + 1).astype(np.float64)
    dinv = (1.0 / np.sqrt(deg)).astype(np.float32)

    core = d_all // NSH
    local = d_all % NSH
    block = local // BLK
    sbk = block // SBW
    par = (s_all & 1).astype(np.int64)

    cidx = (core * NBLK + block) * 2 + par
    cnt = np.bincount(cidx, minlength=NCORES * NBLK * 2).reshape(NCORES, NBLK, 2)
    tiles = ((cnt + BLK - 1) // BLK).max(axis=0)  # [NBLK, 2] max over cores

    TT, tile_base, seg = _layout(tiles)
    S = TT * BLK

    order = np.lexsort((local, block, par, sbk, core))
    s_s = s_all[order]
    d_s = d_all[order]
    core_s = core[order]
    block_s = block[order]
    h_s = par[order]

    gid = (core_s * NBLK + block_s) * 2 + h_s
    change = np.r_[True, gid[1:] != gid[:-1]]
    gstart = np.maximum.accumulate(np.where(change, np.arange(len(gid)), 0))
    rank = np.arange(len(gid)) - gstart
    slot = tile_base[block_s, h_s] * BLK + rank  # per-core slot in [0, S)

    src_loc = (_newpos(s_s & ~1) >> 1).astype(np.int16)  # permuted pair idx
    dst_loc = (d_s % NSH - block_s * BLK).astype(np.float32)  # 0..127
    norm = dinv[d_s] * dinv[s_s]

    seg_slot0_tbl = np.zeros(TT, dtype=np.int64)  # per tile: slot0 of its seg
    for _key, (t0, nt) in seg.items():
        seg_slot0_tbl[t0 : t0 + nt] = t0 * BLK

    meta_np = np.zeros((NCORES, 128, TT, 2), dtype=np.float32)
    idx_np = np.zeros((NCORES, 128, S // 16), dtype=np.int16)
    for c in range(NCORES):
        m = core_s == c
        sl = slot[m]
        tt = sl // BLK
        pp = sl % BLK
        meta_np[c, pp, tt, 0] = dst_loc[m]
        meta_np[c, pp, tt, 1] = norm[m]
        seg0 = seg_slot0_tbl[tt]
        j = sl - seg0
        col = seg0 // 16 + j // 16
        row = j % 16
        v = src_loc[m]
        for g in range(8):  # replicate across the 8 gpsimd 16-partition groups
            idx_np[c, row + 16 * g, col] = v
    allpos = _newpos(np.arange(N, dtype=np.int64))
    order_nodes = np.empty(N, dtype=np.int64)
    order_nodes[allpos] = np.arange(N, dtype=np.int64)
    return tiles, TT, tile_base, seg, meta_np, idx_np, dinv, order_nodes


def _build(prep):
    tiles, TT, tile_base, seg, _meta, _idx, _dinv, _ord = prep
    S = TT * BLK
    f32 = mybir.dt.float32
    bf16 = mybir.dt.bfloat16
    i16 = mybir.dt.int16
    AF = mybir.ActivationFunctionType
    OP = mybir.AluOpType

    nc = bacc.Bacc(
        "TRN2",
        target_bir_lowering=False,
        debug=False,
        num_devices=NCORES,
        num_swdge_queues=NQ,
    )
    xrow = nc.dram_tensor("xrow", [N, FIN], bf16, kind="ExternalInput")
    w1 = nc.dram_tensor("w1", [128, HID], bf16, kind="ExternalInput")
    w2 = nc.dram_tensor("w2", [128, FOUT], bf16, kind="ExternalInput")
    b1c = nc.dram_tensor("b1c", [128, 1], f32, kind="ExternalInput")
    b2r = nc.dram_tensor("b2r", [128, FOUT], f32, kind="ExternalInput")
    iot = nc.dram_tensor("iot", [128, BLK], bf16, kind="ExternalInput")
    meta = nc.dram_tensor("meta", [128, TT, 2], f32, kind="ExternalInput")
    idxt = nc.dram_tensor("idx", [128, S // 16], i16, kind="ExternalInput")
    xself = nc.dram_tensor("xself", [128, NSHP], bf16, kind="ExternalInput")
    ident = nc.dram_tensor("ident", [128, BLK], bf16, kind="ExternalInput")
    onesb = nc.dram_tensor("onesb", [1, BLK], bf16, kind="ExternalInput")
    b2b = nc.dram_tensor("b2b", [1, FOUT], bf16, kind="ExternalInput")
    diag2 = nc.dram_tensor("diag2", [128, NBLK, BLK], bf16, kind="ExternalInput")
    outp = nc.dram_tensor("out", [NSH, FOUT], f32, kind="ExternalOutput")

    with tile.TileContext(nc) as tc, ExitStack() as ctx:
        const = ctx.enter_context(tc.tile_pool(name="const", bufs=1))
        dram = ctx.enter_context(tc.tile_pool(name="dram", bufs=1, space="DRAM"))
        gp1 = ctx.enter_context(tc.tile_pool(name="g1", bufs=2))
        gp2 = ctx.enter_context(tc.tile_pool(name="g2", bufs=2))
        tpp = ctx.enter_context(tc.tile_pool(name="tp", bufs=40))
        evp = ctx.enter_context(tc.tile_pool(name="ev", bufs=4))
        psa = ctx.enter_context(tc.tile_pool(name="psa", bufs=3, space="PSUM"))
        psb = ctx.enter_context(tc.tile_pool(name="psb", bufs=1, space="PSUM"))
        psc = ctx.enter_context(tc.tile_pool(name="psc", bufs=1, space="PSUM"))
        psd = ctx.enter_context(tc.tile_pool(name="psd", bufs=3, space="PSUM"))

        def cload(ap, shape, dtype, tag):
            t = const.tile(shape, dtype, tag=tag)
            nc.sync.dma_start(t[:], ap)
            return t

        idx_sb = cload(idxt[:, :], [128, S // 16], i16, "idx")
        iota_sb = cload(iot[:, :], [128, BLK], bf16, "iota")
        meta_sb = cload(meta[:, :, :], [128, TT, 2], f32, "meta")
        w1_sb = cload(w1[:, :], [128, HID], bf16, "w1")
        w2_sb = cload(w2[:, :], [128, FOUT], bf16, "w2")
        b1_sb = cload(b1c[:, :], [128, 1], f32, "b1")
        b2_sb = cload(b2r[:, :], [128, FOUT], f32, "b2")
        xself_sb = cload(xself[:, :], [128, NSHP], bf16, "xself")
        ident_sb = cload(ident[:, :], [128, BLK], bf16, "ident")
        ones_sb = cload(onesb[:, :], [1, BLK], bf16, "ones")
        b2b_sb = cload(b2b[:, :], [1, FOUT], bf16, "b2b")
        diag_sb = cload(diag2[:, :, :], [128, NBLK, BLK], bf16, "diag2")

        t2loc = dram.tile([NSHP, FOUT], bf16, tag="t2loc")
        t2full = dram.tile([N, FOUT], bf16, tag="t2full")
        # pair view for the layer-2 gather: row p = nodes (2p, 2p+1), 256B
        t2pair = t2full[:, :].rearrange("(p two) f -> p (two f)", two=2)
        # even/odd views of X for the layer-1 gather (elem_step = 2 rows)
        xpar = xrow[:, :].rearrange("(p two) f -> p two f", two=2)

        # zero-fill the padded t2loc tail (layer-2 diag matmul reads it)
        zt = evp.tile([128, FOUT], bf16, tag="zt")
        nc.vector.memset(zt[:], 0)
        nc.sync.dma_start(t2loc[NSH:NSHP, :], zt[0 : NSHP - NSH, :])

        qctr = [0]

        def gather(g, view, t0, ntl, elem, elem_step=None):
            for q0 in range(0, ntl, GCH):
                qn = min(GCH, ntl - q0)
                c0 = (t0 + q0) * 8  # idx columns (tile*128/16)
                nc.gpsimd.dma_gather(
                    out_ap=g[:, q0 : q0 + qn, :],
                    in_ap=view,
                    idxs_ap=idx_sb[:, c0 : c0 + qn * 8],
                    num_idxs=qn * 128,
                    num_idxs_reg=qn * 128,
                    elem_size=elem,
                    elem_step=elem_step,
                    queue_num=qctr[0] % NQ,
                )
                qctr[0] += 1

        def onehot(t):
            tp = tpp.tile([128, BLK], bf16, tag="tp")
            nc.vector.tensor_scalar(
                out=tp[:],
                in0=iota_sb[:],
                scalar1=meta_sb[:, t, 0:1],
                scalar2=meta_sb[:, t, 1:2],
                op0=OP.is_equal,
                op1=OP.mult,
            )
            return tp

        # ---- Layer 1: aggregate X, project W1 (+b1, relu) and W2 per block.
        # Projection of block b-1 is emitted after block b's aggregation so
        # the in-order PE/Act engines never stall at block boundaries.
        def proj_l1(b, a2):
            r0 = b * BLK
            r1 = min(NSH, r0 + BLK)
            poT = psb.tile([128, BLK], f32, tag="psb")  # [hid, dst]
            nc.tensor.matmul(poT[:], lhsT=w1_sb[:], rhs=a2[:], start=True, stop=True)
            hbT = evp.tile([128, BLK], bf16, tag="hbT")
            nc.scalar.activation(hbT[:], poT[:], AF.Relu, bias=b1_sb[:, 0:1])
            t2w = psc.tile([128, FOUT], f32, tag="psc")  # [dst, fout]
            nc.tensor.matmul(t2w[:], lhsT=hbT[:], rhs=w2_sb[:], start=True, stop=True)
            t2e = evp.tile([128, FOUT], bf16, tag="t2e")
            nc.scalar.activation(t2e[:], t2w[:], AF.Copy)
            nc.sync.dma_start(t2loc[r0:r1, :], t2e[0 : r1 - r0, :])

        pend1 = None
        for sb in range(NSB):
            blocks = list(range(sb * SBW, min((sb + 1) * SBW, NBLK)))
            gt = {}
            for h in (0, 1):
                t0, ntl = seg[(sb, h)]
                if ntl == 0:
                    continue
                g = gp1.tile([128, ntl, FIN], bf16, tag=f"g1{h}")
                if ABLATE >= 1:
                    gather(g, xpar[:, h, :], t0, ntl, FIN, elem_step=2 * FIN)
                else:
                    nc.vector.memset(g[:], 0)
                gt[h] = g
            for b in blocks:
                r0 = b * BLK
                ps2 = psa.tile([128, BLK], f32, tag="psa")  # [xfeat, dst]
                # self-loop diagonal: identity lhsT copies the prescaled block
                nc.tensor.matmul(
                    ps2[:],
                    lhsT=ident_sb[:],
                    rhs=xself_sb[:, r0 : r0 + BLK],
                    start=True,
                    stop=False,
                )
                nmm = sum(int(tiles[b][h]) for h in (0, 1))
                if nmm == 0:
                    nc.tensor.matmul(
                        ps2[:], lhsT=ident_sb[:, 0:1], rhs=xself_sb[:, r0 : r0 + 1],
                        start=False, stop=True,
                    )
                done = 0
                for h in (0, 1):
                    seg_t0 = seg[(sb, h)][0]
                    for k in range(int(tiles[b][h])):
                        t = int(tile_base[b][h]) + k
                        gofs = t - seg_t0
                        tp = onehot(t)
                        done += 1
                        nc.tensor.matmul(
                            ps2[:],
                            lhsT=gt[h][:, gofs, :],
                            rhs=tp[:],
                            start=False,
                            stop=done == nmm,
                        )
                a2 = evp.tile([128, BLK], bf16, tag="a2")
                nc.scalar.activation(a2[:], ps2[:], AF.Copy)
                if pend1 is not None:
                    proj_l1(*pend1)
                pend1 = (b, a2)
        proj_l1(*pend1)

        # ---- share the 64-wide t2 table (chunked to overlap layer 1);
        # t2full rows are in (chunk, core, row) order so each chunk's
        # output is one contiguous slice.
        for q, (r0, r1) in enumerate(CCHUNKS):
            g0 = int(_CHB[q])
            g1 = int(_CHB[q + 1])
            if ABLATE >= 2:
                nc.gpsimd.collective_compute(
                    "AllGather",
                    mybir.AluOpType.bypass,
                    replica_groups=[list(range(NCORES))],
                    ins=[t2loc[r0:r1, :].opt()],
                    outs=[t2full[g0:g1, :].opt()],
                )
            else:
                nc.sync.dma_start(t2full[g0 : g0 + (r1 - r0), :], t2loc[r0:r1, :])

        # ---- Layer 2: aggregate t2 pair rows (self diag via dense block).
        # b2 is added inside the psum by a K=1 outer-product matmul; the Act
        # engine (idle in layer 2) evicts, keeping DVE a pure one-hot builder.
        def evict_l2(b, ps):
            r0 = b * BLK
            r1 = min(NSH, r0 + BLK)
            oo = evp.tile([128, FOUT], f32, tag="oo")
            nc.scalar.activation(oo[:], ps[:], AF.Copy)
            nc.sync.dma_start(outp[r0:r1, :], oo[0 : r1 - r0, :])

        pend2 = None
        for sb in range(NSB):
            blocks = list(range(sb * SBW, min((sb + 1) * SBW, NBLK)))
            t0sb = seg[(sb, 0)][0]
            ntlsb = seg[(sb, 0)][1] + seg[(sb, 1)][1]
            g = gp2.tile([128, ntlsb, 2 * FOUT], bf16, tag="g2")
            if ABLATE >= 1:
                gather(g, t2pair, t0sb, ntlsb, 2 * FOUT)
            else:
                nc.vector.memset(g[:], 0)
            gt = {0: g, 1: g}
            for b in blocks:
                r0 = b * BLK
                t2s = evp.tile([128, FOUT], bf16, tag="t2s")
                nc.scalar.dma_start(t2s[:], t2loc[r0 : r0 + BLK, :])
                ps = psd.tile([128, FOUT], f32, tag="psd")  # [dst, fout]
                nc.tensor.matmul(
                    ps[:], lhsT=diag_sb[:, b, :], rhs=t2s[:], start=True, stop=False
                )
                nmm = sum(int(tiles[b][h]) for h in (0, 1))
                nc.tensor.matmul(
                    ps[:], lhsT=ones_sb[0:1, :], rhs=b2b_sb[0:1, :],
                    start=False, stop=nmm == 0,
                )
                done = 0
                for h in (0, 1):
                    for k in range(int(tiles[b][h])):
                        t = int(tile_base[b][h]) + k
                        gofs = t - t0sb
                        tp = onehot(t)
                        done += 1
                        nc.tensor.matmul(
                            ps[:],
                            lhsT=tp[:],
                            rhs=gt[h][:, gofs, h * FOUT : (h + 1) * FOUT],
                            start=False,
                            stop=done == nmm,
                        )
                if pend2 is not None:
                    evict_l2(*pend2)
                pend2 = (b, ps)
        evict_l2(*pend2)

    nc.finalize()
    return nc


def _in_maps(x, W1, b1, W2, b2, prep):
    tiles, TT, tile_base, seg, meta_np, idx_np, dinv, order_nodes = prep
    xf = np.asarray(x, np.float32)
    xr = np.ascontiguousarray(xf[order_nodes].astype(BF16))
    w1b = np.asarray(W1, np.float32).astype(BF16)
    w2b = np.asarray(W2, np.float32).astype(BF16)
    b1col = np.asarray(b1, np.float32).reshape(128, 1).copy()
    b2rep = np.broadcast_to(np.asarray(b2, np.float32), (128, FOUT)).copy()
    iota = np.broadcast_to(np.arange(BLK, dtype=np.float32), (128, BLK)).astype(BF16)
    shared = {
        "xrow": xr,
        "w1": w1b,
        "w2": w2b,
        "b1c": b1col,
        "b2r": b2rep,
        "iot": np.ascontiguousarray(iota),
        "ident": np.ascontiguousarray(np.eye(BLK, dtype=np.float32).astype(BF16)),
        "onesb": np.ones((1, BLK), dtype=BF16),
        "b2b": np.asarray(b2, np.float32).reshape(1, FOUT).astype(BF16),
    }
    d2 = (dinv.astype(np.float64) ** 2).astype(np.float32)
    maps = []
    for c in range(NCORES):
        lo, hi = c * NSH, (c + 1) * NSH
        xs = np.zeros((128, NSHP), dtype=BF16)
        xs[:, :NSH] = (xf[lo:hi] * d2[lo:hi, None]).T.astype(BF16)
        dvals = np.zeros(NSHP, np.float32)
        dvals[:NSH] = d2[lo:hi]
        dg = np.zeros((128, NBLK, BLK), dtype=np.float32)
        for b in range(NBLK):
            dg[np.arange(BLK), b, np.arange(BLK)] = dvals[b * BLK : (b + 1) * BLK]
        maps.append(
            dict(
                shared,
                meta=np.ascontiguousarray(meta_np[c]),
                idx=np.ascontiguousarray(idx_np[c]),
                xself=xs,
                diag2=np.ascontiguousarray(dg.astype(BF16)),
            )
        )
    return maps


def kernel(x, edge_index, W1, b1, W2, b2):
    prep = _prep(edge_index)
    nc = _build(prep)
    in_maps = _in_maps(x, W1, b1, W2, b2, prep)
    res = run_bass_kernel_spmd(nc, in_maps, core_ids=list(range(NCORES)), trace=False)
    out = np.concatenate(
        [res.results[c]["out"].astype(np.float32) for c in range(NCORES)], axis=0
    )
    return out
